# revision 12
# baseline (speedup 1.0000x reference)
"""Trainium2 Bass kernel for nn_BasicTT (TT-decomposed 3-layer MLP + log_softmax).

Strategy (8-way batch data parallelism, b=256 per core):
  Host prep (numpy):
    - Merge layer-1 TT cores 3,4,5 -> lhsT_A [K=512=(n3,n4,n5), 128=(r2,m3,m4,m5)]
    - Merge layer-1 cores 1,2 (+ layer-1 bias in pad rows) -> lhsT_B [128, 64]
    - Layer 2 and 3 TT weights densified: g2 [128,(v16,64)], g3 [64,32]
    - Final linear reduced to the logit difference d = (W[1]-W[0])@h3 + bld
      (bld folded as a 33rd row of the d-matmul against a ones-row of h3);
      log_softmax = [-softplus(d), d - softplus(d)]
    - x pre-transposed per core to xT [512, b*24] fp16; all consts in one
      fp16 DMA
  Device (per core), pipelined per batch chunk (b16 head/tail chunks for
  low DMA-completion latency at the kernel edges, b32 in the middle):
    - warmup matmuls on dummy data span the HAM window while the input
      DMA streams, so real work starts at full PE clock
    - Phase A: accumulating K=128 fp16 matmuls -> psum [128,(b16,24)]
    - ScalarE copies psum -> sg [128,(b,32)] (pad cols hold the persistent
      bias-delta pattern); one DVE 32x32 stream-transpose per chunk
    - Phase B: per-b16 matmul (bias via pad rows) -> psB [64,(b16,u32)]
    - relu-split into h1 [128,(b128,16)]: ScalarE takes the aligned half,
      DVE the partition-shifted half
    - per b128 half: L2 as 16 acc matmuls alternating between two PSUM
      banks (even/odd v) to avoid drain-to-fill serialization, summed on
      DVE -> relu+b2 -> L3 -> relu+b3 -> d-matmul -> sigmoid-series
      softplus tail -> contiguous y DMA
  Only Abs/Sigmoid/Square/Relu/Copy activations are used (single ACT
  table load, pinned to kernel start by a dummy Sigmoid).
"""
import os
import numpy as np

NCORES = 8
B = 2048
BLOC = B // NCORES  # 256
# chunk sizes: small edges (fast DMA completion), b32 middle
CHUNKS = [16, 16, 32, 32, 32, 32, 32, 32, 16, 16]
assert sum(CHUNKS) == BLOC

_prog_cache = {}


# ---------------------------------------------------------------------------
# Host-side weight preparation
# ---------------------------------------------------------------------------
def _tt_full_matrix(cores):
    """Dense matrix W [prod(m), prod(n)] of a TT layer, matching the
    reference tt_linear index convention."""
    n = 1
    for G in cores:
        n *= G.shape[2]
    x = np.eye(n)
    b = n
    z = x.reshape(b, 1, -1)
    for G in cores:
        r0, m, nn_, r1 = G.shape
        z4 = z.reshape(b, r0, nn_, -1)
        z = np.einsum('brns,rmnq->bqsm', z4, G).reshape(b, r1, -1)
    return z.reshape(b, -1).T


def _build_host_tensors(p):
    f64 = {k: np.asarray(v, np.float64) for k, v in p.items()}

    # G345 = l1c2 (r2,m3,n3,r3) * l1c3 (r3,m4,n4,r4) * l1c4 (r4,m5,n5,1)
    g34 = np.einsum('amcb,bndq->amncdq', f64['l1c2'], f64['l1c3'])
    g345 = np.einsum('amncdq,qpe->amnpcde', g34, f64['l1c4'][:, :, :, 0])
    # g345[r2,m3,m4,m5,n3,n4,n5] -> lhsT_A [(n3,n4,n5)=512, (r2,m3,m4,m5)=128]
    lhsT_A = g345.transpose(4, 5, 6, 0, 1, 2, 3).reshape(512, 128)
    gA = np.ascontiguousarray(
        lhsT_A.reshape(4, 128, 128).transpose(1, 0, 2).reshape(128, 512))

    # G12 = l1c0 (1,m1,n1,r1) * l1c1 (r1,m2,n2,r2) -> g12[n1,n2,r2,m1,m2]
    g12 = np.einsum('mar,rnbq->abqmn', f64['l1c0'][0], f64['l1c1'])
    b1 = f64['b1']  # (m1,m2,m3,m4,m5) = (8,4,4,4,4)
    lhsT_B = np.zeros((128, 64))
    for r2 in range(2):
        for m3h in range(2):
            g = r2 * 2 + m3h
            for n1 in range(3):
                for n2 in range(8):
                    j = n1 * 8 + n2
                    for m1 in range(8):
                        for m2 in range(4):
                            lhsT_B[g * 32 + j, m3h * 32 + m1 * 4 + m2] = \
                                g12[n1, n2, r2, m1, m2]
    # bias rows: row (g, 24+j') fires for u = 8g+j'
    for g in range(4):
        for jp in range(8):
            u = 8 * g + jp
            m3l, m4, m5 = u >> 4, (u >> 2) & 3, u & 3
            for m3h in range(2):
                for m1 in range(8):
                    for m2 in range(4):
                        m3 = m3h * 2 + m3l
                        lhsT_B[g * 32 + 24 + jp, m3h * 32 + m1 * 4 + m2] = \
                            b1[m1, m2, m3, m4, m5]
    # delta pattern for the sg pads, tiled over b32: row p=(g,u) has 1.0 at
    # pad col (b, j') iff u == 8g+j'
    dltrep = np.zeros((128, 8))
    for gg in range(4):
        for u in range(32):
            jp = u - 8 * gg
            if 0 <= jp < 8:
                dltrep[gg * 32 + u, jp] = 1.0
    dltrep = np.tile(dltrep, (1, 32))  # [128, 256]

    # dense layer 2/3
    W2 = _tt_full_matrix([f64['l2c0'], f64['l2c1'], f64['l2c2'],
                          f64['l2c3'], f64['l2c4']])  # [64, 2048]
    W3 = _tt_full_matrix([f64['l3c0'], f64['l3c1'], f64['l3c2'],
                          f64['l3c3'], f64['l3c4']])  # [32, 64]
    # g2 [128=(m3l,m3h,m1,m2), (v16, f64)]
    g2 = np.zeros((128, 16, 64))
    for pp in range(128):
        m3l, m3h = pp >> 6, (pp >> 5) & 1
        m1, m2 = (pp >> 2) & 7, pp & 3
        m3 = m3h * 2 + m3l
        for v in range(16):
            m4, m5 = v >> 2, v & 3
            flat = (((m1 * 4 + m2) * 4 + m3) * 4 + m4) * 4 + m5
            g2[pp, v, :] = W2[:, flat]
    g2 = g2.reshape(128, 1024)

    # all consts fp16, one DMA:
    #   gA 0:512 | g2 512:1536 | gB 1536:1600 | g3 1600:1632 (rows 0:64)
    #   | wd 1632:1633 (rows 0:33; row 32 = bld) | dltrep 1633:1889
    #   | b2 1889:1890 (rows 0:64) | b3 1890:1891 (rows 0:32)
    cstH = np.zeros((128, 1891), np.float16)
    cstH[:, 0:512] = gA.astype(np.float16)
    cstH[:, 512:1536] = g2.astype(np.float16)
    cstH[:, 1536:1600] = lhsT_B.astype(np.float16)
    cstH[0:64, 1600:1632] = W3.T.astype(np.float16)
    cstH[0:32, 1632:1633] = (f64['W'][1] - f64['W'][0]).reshape(32, 1) \
        .astype(np.float16)
    cstH[32, 1632] = np.float16(f64['bl'][1] - f64['bl'][0])
    cstH[:, 1633:1889] = dltrep.astype(np.float16)
    cstH[0:64, 1889:1890] = f64['b2'].reshape(64, 1).astype(np.float16)
    cstH[0:32, 1890:1891] = f64['b3'].reshape(32, 1).astype(np.float16)
    return dict(cstH=cstH)


def _make_xT(x_core):
    b = x_core.shape[0]
    xr = np.asarray(x_core, np.float32).reshape(b, 24, 512)
    xt = np.ascontiguousarray(xr.transpose(2, 0, 1).reshape(512, b * 24))
    return xt.astype(np.float16)


# ---------------------------------------------------------------------------
# Device program
# ---------------------------------------------------------------------------
def _build_program():
    if 'nc' in _prog_cache:
        return _prog_cache['nc']
    from contextlib import ExitStack
    import concourse.bacc as bacc
    import concourse.mybir as mybir
    import concourse.tile as tile

    F16 = mybir.dt.float16
    F32 = mybir.dt.float32
    AF = mybir.ActivationFunctionType
    MUL = mybir.AluOpType.mult
    ADD = mybir.AluOpType.add
    SUB = mybir.AluOpType.subtract

    nc = bacc.Bacc(None, target_bir_lowering=False)

    xT = nc.declare_dram_parameter("xT", [512, BLOC * 24], F16, isOutput=False)
    cstH = nc.declare_dram_parameter("cstH", [128, 1891], F16, isOutput=False)
    y = nc.declare_dram_parameter("y", [BLOC, 2], F32, isOutput=True)

    with tile.TileContext(nc) as tc, ExitStack() as ctx:
        consts = ctx.enter_context(tc.tile_pool(name="consts", bufs=1))
        xpool = ctx.enter_context(tc.tile_pool(name="x", bufs=3))
        tpool = ctx.enter_context(tc.tile_pool(name="tb", bufs=3))
        h1pool = ctx.enter_context(tc.tile_pool(name="h1", bufs=1))
        spool = ctx.enter_context(tc.tile_pool(name="small", bufs=1))
        psA = ctx.enter_context(tc.tile_pool(name="psA", bufs=4, space="PSUM"))
        psB = ctx.enter_context(tc.tile_pool(name="psB", bufs=2, space="PSUM"))
        psT = ctx.enter_context(tc.tile_pool(name="psT", bufs=2, space="PSUM"))

        cH = consts.tile([128, 1891], F16, tag="cstH")
        nc.sync.dma_start(cH[:, :], cstH[:, :])
        gA_t = cH[:, 0:512]
        g2_t = cH[:, 512:1536]
        gB_t = cH[:, 1536:1600]
        g3_t = cH[0:64, 1600:1632]
        wd_t = cH[0:33, 1632:1633]
        dltrep_t = cH[:, 1633:1889]
        b2_t = cH[0:64, 1889:1890]
        b3_t = cH[0:32, 1890:1891]

        # PE warmup during the input-DMA window: dummy matmuls on a zeroed
        # tile span the HAM activity window so real work runs at 2.4 GHz.
        wrm = consts.tile([128, 512], F16, tag="wrm")
        nc.vector.memset(wrm[:, :], 0.0)
        # dummy Sigmoid first: every activation used later (Abs, Sigmoid,
        # Square, Relu, Copy) lives in the sigmoid table, so this pins the
        # one and only ACT table load to kernel start
        sgd = consts.tile([1, 1], F32, tag="sgd")
        nc.scalar.activation(sgd[:, :], wrm[0:1, 0:1], AF.Sigmoid)
        for w in range(8):
            pw = psA.tile([128, 384], F32, tag="psA", name=f"warm{w}")
            nc.tensor.matmul(pw[:, :], wrm[:, 0:128], wrm[:, 0:384],
                             start=True, stop=True)

        # persistent staging buffers: pad cols (b, 24:32) hold the bias
        # delta pattern, written once; data cols rewritten every chunk
        sg_bufs = []
        for i in range(3):
            z = tpool.tile([128, 32 * 32], F16, tag=f"sg{i}", name=f"sg{i}")
            nc.vector.tensor_copy(
                z.rearrange("p (b j) -> p b j", j=32)[:, :, 24:32],
                dltrep_t.rearrange("p (b j) -> p b j", j=8))
            sg_bufs.append(z)

        # h1 per b128 half [128=(m3l,m3h,m1,m2), (b128, v16)]
        h1h = [h1pool.tile([128, 128 * 16], F16, tag=f"h1{h}", name=f"h1{h}")
               for h in range(2)]
        # h3 per half [33, 128]: row 32 is the ones-row that folds bld in
        h3h = []
        for h in range(2):
            t = spool.tile([33, 128], F16, tag=f"h3{h}", name=f"h3{h}")
            nc.gpsimd.memset(t[32:33, :], 1.0)
            h3h.append(t)
        # y staging [1, (b256, i2)] f32, one contiguous DMA per half
        ystage = consts.tile([1, 512], F32, tag="ystage")

        b0 = 0
        for ci, bc in enumerate(CHUNKS):
            half = b0 // 128
            nhf = bc // 16
            # xt layout [128, (k4, bc, 24)]: fully contiguous DMA
            xt = xpool.tile([128, 4 * 32 * 24], F16, tag="xt", name=f"xt{ci}")
            nc.sync.dma_start(
                xt[:, 0:4 * bc * 24].rearrange("p (k c) -> p k c", k=4),
                xT.rearrange("(k p) c -> p k c", k=4)
                [:, :, b0 * 24:(b0 + bc) * 24])
            xt4 = xt[:, 0:4 * bc * 24].rearrange("p (k b j) -> p k b j",
                                                 k=4, j=24)
            pss = [psA.tile([128, 384], F32, tag="psA", name=f"psA{ci}_{hf}")
                   for hf in range(nhf)]
            for k in range(4):  # weights loaded once per k
                for hf in range(nhf):
                    nc.tensor.matmul(
                        pss[hf][:, :],
                        gA_t[:, k * 128:(k + 1) * 128],
                        xt4[:, k, hf * 16:(hf + 1) * 16, :],
                        start=(k == 0), stop=(k == 3))
            # stage psum->sbuf (ScalarE) with (b,32) interleave; pad cols
            # already hold the delta pattern (bias rows of gB)
            sg = sg_bufs[ci % 3]
            sg3 = sg.rearrange("p (b j) -> p b j", j=32)
            for hf in range(nhf):
                nc.scalar.activation(
                    sg3[:, hf * 16:(hf + 1) * 16, 0:24],
                    pss[hf].rearrange("p (b j) -> p b j", j=24),
                    AF.Copy)
            # 32x32 stream transpose: [(g,u),(b,j)] -> [(g,j),(b,u)]
            tb = tpool.tile([128, 32 * 32], F16, tag="tb", name=f"tb{ci}")
            nc.vector.transpose(tb[:, 0:bc * 32], sg[:, 0:bc * 32])
            lb = b0 - half * 128
            dst = h1h[half][:, lb * 16:(lb + bc) * 16] \
                .rearrange("p (b v) -> p b v", v=16)
            for hf in range(nhf):
                # phase B: layer-1 left side + bias (pad rows x delta)
                pb = psB.tile([64, 512], F32, tag="psB", name=f"psB{ci}_{hf}")
                nc.tensor.matmul(pb[:, :], gB_t[:, :],
                                 tb[:, hf * 512:(hf + 1) * 512],
                                 start=True, stop=True)
                # relu + split (m3l) into h1: aligned half on ScalarE,
                # partition-shifted half on DVE
                pb3 = pb.rearrange("p (b u) -> p b u", u=32)
                d3 = dst[:, hf * 16:(hf + 1) * 16, :]
                nc.scalar.activation(d3[0:64], pb3[:, :, 0:16], AF.Relu)
                nc.vector.tensor_scalar_max(d3[64:128], pb3[:, :, 16:32], 0.0)
            b0 += bc

            if b0 % 128 == 0:
                # ---- layers 2/3 + log_softmax tail for this b128 half ----
                # L2: even/odd v alternate between two PSUM banks so the
                # accumulating matmuls pipeline instead of serializing
                p2 = [psT.tile([64, 128], F32, tag="pt", name=f"p2{half}_{e}")
                      for e in range(2)]
                h1v = h1h[half].rearrange("p (b v) -> p v b", v=16)
                for v in range(16):
                    nc.tensor.matmul(p2[v % 2][:, :],
                                     g2_t[:, v * 64:(v + 1) * 64],
                                     h1v[:, v, :], start=(v < 2),
                                     stop=(v >= 14))
                s2a = spool.tile([64, 128], F32, tag=f"s2a{half}",
                                 name=f"s2a{half}")
                nc.scalar.activation(s2a[:, :], p2[0][:, :], AF.Copy)
                s2s = spool.tile([64, 128], F32, tag=f"s2s{half}",
                                 name=f"s2s{half}")
                nc.vector.tensor_tensor(s2s[:, :], s2a[:, :], p2[1][:, :],
                                        op=ADD)
                h2 = spool.tile([64, 128], F16, tag=f"h2{half}",
                                name=f"h2_{half}")
                nc.scalar.activation(h2[:, :], s2s[:, :], AF.Relu,
                                     bias=b2_t[:, 0:1])
                p3 = psT.tile([32, 128], F32, tag="pt", name=f"p3_{half}")
                nc.tensor.matmul(p3[:, :], g3_t[:, :], h2[:, :],
                                 start=True, stop=True)
                nc.scalar.activation(h3h[half][0:32, :], p3[:, :], AF.Relu,
                                     bias=b3_t[:, 0:1])
                # logit diff (K=33 includes the bld ones-row)
                pd = psT.tile([1, 128], F32, tag="pt", name=f"pd_{half}")
                nc.tensor.matmul(pd[:, :], wd_t[:, :], h3h[half][:, :],
                                 start=True, stop=True)
                # log_softmax = [-softplus(d), d - softplus(d)] with
                # softplus(d) = relu(d) - ln(1-s), s = sigmoid(-|d|), and
                # -ln(1-s) ~= s + s^2/2 + s^3/3 + s^4/4 + s^5/5 (|err|<.5%)
                #           = s*(1 + s2/3 + s4/5) + s2*(1/2 + s2/4)
                aT = spool.tile([1, 128], F32, tag=f"aT{half}",
                                name=f"aT{half}")
                nc.scalar.activation(aT[:, :], pd[:, :], AF.Abs)
                sS = spool.tile([1, 128], F32, tag=f"sS{half}",
                                name=f"sS{half}")
                nc.scalar.activation(sS[:, :], aT[:, :], AF.Sigmoid,
                                     scale=-1.0)
                s2 = spool.tile([1, 128], F32, tag=f"s2{half}",
                                name=f"s2{half}")
                nc.scalar.activation(s2[:, :], sS[:, :], AF.Square)
                s4 = spool.tile([1, 128], F32, tag=f"s4{half}",
                                name=f"s4{half}")
                nc.scalar.activation(s4[:, :], s2[:, :], AF.Square)
                rl = spool.tile([1, 128], F32, tag=f"rl{half}",
                                name=f"rl{half}")
                nc.scalar.activation(rl[:, :], pd[:, :], AF.Relu)
                qa = spool.tile([1, 128], F32, tag=f"qa{half}",
                                name=f"qa{half}")
                qb = spool.tile([1, 128], F32, tag=f"qb{half}",
                                name=f"qb{half}")
                qc = spool.tile([1, 128], F32, tag=f"qc{half}",
                                name=f"qc{half}")
                # qa = 1 + s2/3 + s4/5 ; qb = 1/2 + s2/4
                nc.vector.tensor_scalar(qa[:, :], s4[:, :], 0.2, 1.0,
                                        MUL, ADD)
                nc.vector.scalar_tensor_tensor(qa[:, :], s2[:, :], 1.0 / 3,
                                               qa[:, :], MUL, ADD)
                nc.vector.tensor_scalar(qb[:, :], s2[:, :], 0.25, 0.5,
                                        MUL, ADD)
                # qc = s*qa + s2*qb = -ln(1-s);  then y0 = -(qc + relu(d))
                nc.vector.tensor_tensor(qc[:, :], sS[:, :], qa[:, :], op=MUL)
                nc.vector.tensor_tensor(qb[:, :], s2[:, :], qb[:, :], op=MUL)
                nc.vector.tensor_tensor(qc[:, :], qc[:, :], qb[:, :], op=ADD)
                yv = ystage.rearrange("p (b i) -> p b i", i=2)
                hb = half * 128
                nc.vector.scalar_tensor_tensor(
                    yv[:, hb:hb + 128, 0:1],
                    qc.rearrange("p (f o) -> p f o", o=1), -1.0,
                    rl.rearrange("p (f o) -> p f o", o=1),
                    MUL, SUB)
                nc.vector.tensor_tensor(
                    yv[:, hb:hb + 128, 1:2],
                    pd.rearrange("p (f o) -> p f o", o=1),
                    yv[:, hb:hb + 128, 0:1], op=ADD)
                nc.sync.dma_start(
                    y.rearrange("(h b) i -> h (b i)", h=2)[half:half + 1, :],
                    ystage[:, hb * 2:hb * 2 + 256])

    nc.compile()
    _prog_cache['nc'] = nc
    return nc


# ---------------------------------------------------------------------------
# Entry point
# ---------------------------------------------------------------------------
def kernel(**inputs):
    from concourse.bass_utils import run_bass_kernel_spmd

    H = _build_host_tensors(inputs)
    x = np.asarray(inputs['x'], np.float32)
    nc = _build_program()

    in_maps = []
    for c in range(NCORES):
        m = dict(H)
        m['xT'] = _make_xT(x[c * BLOC:(c + 1) * BLOC])
        in_maps.append(m)

    trace = bool(os.environ.get('KERNEL_TRACE'))
    tmpdir = None
    if trace:
        tmpdir = os.environ.get('KERNEL_TRACE_DIR') or None
        if tmpdir:
            os.makedirs(tmpdir, exist_ok=True)
    res = run_bass_kernel_spmd(nc, in_maps, list(range(NCORES)),
                               trace=trace, tmpdir=tmpdir)
    kernel.last_results = res
    out = np.concatenate([res.results[c]['y'] for c in range(NCORES)], axis=0)
    return out.astype(np.float32)


if __name__ == '__main__':
    # smoke test with random inputs shaped per spec
    rng = np.random.default_rng(0)
    shapes = {
        'x': (B, 3, 8, 8, 8, 8),
        'l1c0': (1, 8, 3, 3), 'l1c1': (3, 4, 8, 2), 'l1c2': (2, 4, 8, 2),
        'l1c3': (2, 4, 8, 2), 'l1c4': (2, 4, 8, 1), 'b1': (8, 4, 4, 4, 4),
        'l2c0': (1, 4, 8, 2), 'l2c1': (2, 2, 4, 2), 'l2c2': (2, 2, 4, 2),
        'l2c3': (2, 2, 4, 2), 'l2c4': (2, 2, 4, 1), 'b2': (4, 2, 2, 2, 2),
        'l3c0': (1, 2, 4, 2), 'l3c1': (2, 2, 2, 2), 'l3c2': (2, 2, 2, 2),
        'l3c3': (2, 2, 2, 2), 'l3c4': (2, 2, 2, 1), 'b3': (2, 2, 2, 2, 2),
        'W': (2, 32), 'bl': (2,),
    }
    ins = {k: rng.standard_normal(v).astype(np.float32) * 0.3
           for k, v in shapes.items()}
    print(kernel(**ins)[:4])


# revision 13
# speedup vs baseline: 1.1114x; 1.1114x over previous
"""Trainium2 Bass kernel for nn_BasicTT (TT-decomposed 3-layer MLP + log_softmax).

Strategy (8-way batch data parallelism, b=256 per core):
  Host prep (numpy):
    - Merge layer-1 TT cores 3,4,5 -> lhsT_A [K=512=(n3,n4,n5), 128=(r2,m3,m4,m5)]
    - Merge layer-1 cores 1,2 (+ layer-1 bias in pad rows) -> lhsT_B [128, 64]
    - Layer 2 and 3 TT weights densified: g2 [128,(v16,64)], g3 [64,32]
    - Final linear reduced to the logit difference d = (W[1]-W[0])@h3 + bld
      (bld folded as a 33rd row of the d-matmul against a ones-row of h3);
      log_softmax = [-softplus(d), d - softplus(d)]
    - x pre-transposed per core to xT [512, b*24] fp16; all consts in one
      fp16 DMA
  Device (per core), pipelined per batch chunk (b16 head/tail chunks for
  low DMA-completion latency at the kernel edges, b32 in the middle):
    - warmup matmuls on dummy data span the HAM window while the input
      DMA streams, so real work starts at full PE clock
    - Phase A: accumulating K=128 fp16 matmuls -> psum [128,(b16,24)]
    - ScalarE copies psum -> sg [128,(b,32)] (pad cols hold the persistent
      bias-delta pattern); one DVE 32x32 stream-transpose per chunk
    - Phase B: per-b16 matmul (bias via pad rows) -> psB [64,(b16,u32)]
    - relu-split into h1 [128,(b128,16)]: ScalarE takes the aligned half,
      DVE the partition-shifted half
    - per b128 half: L2 as 16 acc matmuls alternating between two PSUM
      banks (even/odd v) to avoid drain-to-fill serialization, summed on
      DVE -> relu+b2 -> L3 -> relu+b3 -> d-matmul -> sigmoid-series
      softplus tail -> contiguous y DMA
  Only Abs/Sigmoid/Square/Relu/Copy activations are used (single ACT
  table load, pinned to kernel start by a dummy Sigmoid).
"""
import os
import numpy as np

NCORES = 8
B = 2048
BLOC = B // NCORES  # 256
# chunk sizes: small edges (fast DMA completion), b32 middle
CHUNKS = [16, 16, 32, 32, 32, 32, 32, 32, 16, 16]
assert sum(CHUNKS) == BLOC

_prog_cache = {}


# ---------------------------------------------------------------------------
# Host-side weight preparation
# ---------------------------------------------------------------------------
def _tt_full_matrix(cores):
    """Dense matrix W [prod(m), prod(n)] of a TT layer, matching the
    reference tt_linear index convention."""
    n = 1
    for G in cores:
        n *= G.shape[2]
    x = np.eye(n)
    b = n
    z = x.reshape(b, 1, -1)
    for G in cores:
        r0, m, nn_, r1 = G.shape
        z4 = z.reshape(b, r0, nn_, -1)
        z = np.einsum('brns,rmnq->bqsm', z4, G).reshape(b, r1, -1)
    return z.reshape(b, -1).T


def _build_host_tensors(p):
    f64 = {k: np.asarray(v, np.float64) for k, v in p.items()}

    # G345 = l1c2 (r2,m3,n3,r3) * l1c3 (r3,m4,n4,r4) * l1c4 (r4,m5,n5,1)
    g34 = np.einsum('amcb,bndq->amncdq', f64['l1c2'], f64['l1c3'])
    g345 = np.einsum('amncdq,qpe->amnpcde', g34, f64['l1c4'][:, :, :, 0])
    # g345[r2,m3,m4,m5,n3,n4,n5] -> lhsT_A [(n3,n4,n5)=512, (r2,m3,m4,m5)=128]
    lhsT_A = g345.transpose(4, 5, 6, 0, 1, 2, 3).reshape(512, 128)
    gA = np.ascontiguousarray(
        lhsT_A.reshape(4, 128, 128).transpose(1, 0, 2).reshape(128, 512))

    # G12 = l1c0 (1,m1,n1,r1) * l1c1 (r1,m2,n2,r2) -> g12[n1,n2,r2,m1,m2]
    g12 = np.einsum('mar,rnbq->abqmn', f64['l1c0'][0], f64['l1c1'])
    b1 = f64['b1']  # (m1,m2,m3,m4,m5) = (8,4,4,4,4)
    lhsT_B = np.zeros((128, 64))
    for r2 in range(2):
        for m3h in range(2):
            g = r2 * 2 + m3h
            for n1 in range(3):
                for n2 in range(8):
                    j = n1 * 8 + n2
                    for m1 in range(8):
                        for m2 in range(4):
                            lhsT_B[g * 32 + j, m3h * 32 + m1 * 4 + m2] = \
                                g12[n1, n2, r2, m1, m2]
    # bias rows: row (g, 24+j') fires for u = 8g+j'
    for g in range(4):
        for jp in range(8):
            u = 8 * g + jp
            m3l, m4, m5 = u >> 4, (u >> 2) & 3, u & 3
            for m3h in range(2):
                for m1 in range(8):
                    for m2 in range(4):
                        m3 = m3h * 2 + m3l
                        lhsT_B[g * 32 + 24 + jp, m3h * 32 + m1 * 4 + m2] = \
                            b1[m1, m2, m3, m4, m5]
    # delta pattern for the sg pads, tiled over b32: row p=(g,u) has 1.0 at
    # pad col (b, j') iff u == 8g+j'
    dltrep = np.zeros((128, 8))
    for gg in range(4):
        for u in range(32):
            jp = u - 8 * gg
            if 0 <= jp < 8:
                dltrep[gg * 32 + u, jp] = 1.0
    dltrep = np.tile(dltrep, (1, 32))  # [128, 256]

    # dense layer 2/3
    W2 = _tt_full_matrix([f64['l2c0'], f64['l2c1'], f64['l2c2'],
                          f64['l2c3'], f64['l2c4']])  # [64, 2048]
    W3 = _tt_full_matrix([f64['l3c0'], f64['l3c1'], f64['l3c2'],
                          f64['l3c3'], f64['l3c4']])  # [32, 64]
    # g2 [128=(m3l,m3h,m1,m2), (v16, f64)]
    g2 = np.zeros((128, 16, 64))
    for pp in range(128):
        m3l, m3h = pp >> 6, (pp >> 5) & 1
        m1, m2 = (pp >> 2) & 7, pp & 3
        m3 = m3h * 2 + m3l
        for v in range(16):
            m4, m5 = v >> 2, v & 3
            flat = (((m1 * 4 + m2) * 4 + m3) * 4 + m4) * 4 + m5
            g2[pp, v, :] = W2[:, flat]
    g2 = g2.reshape(128, 1024)

    # consts fp16, two DMAs: cstA holds everything the chunk pipeline
    # needs (so phase A can start as soon as possible), cstB what the
    # per-half tail needs (L2/L3/logit weights)
    #   cstA: gA 0:512 | gB 512:576 | dltrep 576:832 | b2 832:833 | b3 833:834
    #   cstB: g2 0:1024 | g3 1024:1056 (rows 0:64) | wd 1056:1057 (rows 0:33)
    cstA = np.zeros((128, 834), np.float16)
    cstA[:, 0:512] = gA.astype(np.float16)
    cstA[:, 512:576] = lhsT_B.astype(np.float16)
    cstA[:, 576:832] = dltrep.astype(np.float16)
    cstA[0:64, 832:833] = f64['b2'].reshape(64, 1).astype(np.float16)
    cstA[0:32, 833:834] = f64['b3'].reshape(32, 1).astype(np.float16)
    cstB = np.zeros((128, 1057), np.float16)
    cstB[:, 0:1024] = g2.astype(np.float16)
    cstB[0:64, 1024:1056] = W3.T.astype(np.float16)
    cstB[0:32, 1056:1057] = (f64['W'][1] - f64['W'][0]).reshape(32, 1) \
        .astype(np.float16)
    cstB[32, 1056] = np.float16(f64['bl'][1] - f64['bl'][0])
    return dict(cstA=cstA, cstB=cstB)


def _make_xT(x_core):
    b = x_core.shape[0]
    xr = np.asarray(x_core, np.float32).reshape(b, 24, 512)
    xt = np.ascontiguousarray(xr.transpose(2, 0, 1).reshape(512, b * 24))
    return xt.astype(np.float16)


# ---------------------------------------------------------------------------
# Device program
# ---------------------------------------------------------------------------
def _build_program():
    if 'nc' in _prog_cache:
        return _prog_cache['nc']
    from contextlib import ExitStack
    import concourse.bacc as bacc
    import concourse.mybir as mybir
    import concourse.tile as tile

    F16 = mybir.dt.float16
    F32 = mybir.dt.float32
    AF = mybir.ActivationFunctionType
    MUL = mybir.AluOpType.mult
    ADD = mybir.AluOpType.add
    SUB = mybir.AluOpType.subtract

    nc = bacc.Bacc(None, target_bir_lowering=False)

    xT = nc.declare_dram_parameter("xT", [512, BLOC * 24], F16, isOutput=False)
    cstA = nc.declare_dram_parameter("cstA", [128, 834], F16, isOutput=False)
    cstB = nc.declare_dram_parameter("cstB", [128, 1057], F16, isOutput=False)
    y = nc.declare_dram_parameter("y", [BLOC, 2], F32, isOutput=True)

    with tile.TileContext(nc) as tc, ExitStack() as ctx:
        consts = ctx.enter_context(tc.tile_pool(name="consts", bufs=1))
        xpool = ctx.enter_context(tc.tile_pool(name="x", bufs=3))
        tpool = ctx.enter_context(tc.tile_pool(name="tb", bufs=3))
        h1pool = ctx.enter_context(tc.tile_pool(name="h1", bufs=1))
        spool = ctx.enter_context(tc.tile_pool(name="small", bufs=1))
        psA = ctx.enter_context(tc.tile_pool(name="psA", bufs=4, space="PSUM"))
        psB = ctx.enter_context(tc.tile_pool(name="psB", bufs=2, space="PSUM"))
        psT = ctx.enter_context(tc.tile_pool(name="psT", bufs=1, space="PSUM"))

        cA = consts.tile([128, 834], F16, tag="cstA")
        nc.sync.dma_start(cA[:, :], cstA[:, :])
        cB = consts.tile([128, 1057], F16, tag="cstB")
        nc.sync.dma_start(cB[:, :], cstB[:, :])
        gA_t = cA[:, 0:512]
        gB_t = cA[:, 512:576]
        dltrep_t = cA[:, 576:832]
        b2_t = cA[0:64, 832:833]
        b3_t = cA[0:32, 833:834]
        g2_t = cB[:, 0:1024]
        g3_t = cB[0:64, 1024:1056]
        wd_t = cB[0:33, 1056:1057]

        # PE warmup during the input-DMA window: dummy matmuls on a zeroed
        # tile span the HAM activity window so real work runs at 2.4 GHz.
        wrm = consts.tile([128, 512], F16, tag="wrm")
        nc.vector.memset(wrm[:, :], 0.0)
        # dummy Sigmoid first: every activation used later (Abs, Sigmoid,
        # Square, Relu, Copy) lives in the sigmoid table, so this pins the
        # one and only ACT table load to kernel start
        sgd = consts.tile([1, 1], F32, tag="sgd")
        nc.scalar.activation(sgd[:, :], wrm[0:1, 0:1], AF.Sigmoid)
        for w in range(5):
            pw = psA.tile([128, 384], F32, tag="psA", name=f"warm{w}")
            nc.tensor.matmul(pw[:, :], wrm[:, 0:128], wrm[:, 0:384],
                             start=True, stop=True)

        # persistent staging buffers: pad cols (b, 24:32) hold the bias
        # delta pattern, written once; data cols rewritten every chunk
        sg_bufs = []
        for i in range(3):
            z = tpool.tile([128, 32 * 32], F16, tag=f"sg{i}", name=f"sg{i}")
            nc.vector.tensor_copy(
                z.rearrange("p (b j) -> p b j", j=32)[:, :, 24:32],
                dltrep_t.rearrange("p (b j) -> p b j", j=8))
            sg_bufs.append(z)

        # h1 per b128 half [128=(m3l,m3h,m1,m2), (b128, v16)]
        h1h = [h1pool.tile([128, 128 * 16], F16, tag=f"h1{h}", name=f"h1{h}")
               for h in range(2)]
        # h3 per half [33, 128]: row 32 is the ones-row that folds bld in
        h3h = []
        for h in range(2):
            t = spool.tile([33, 128], F16, tag=f"h3{h}", name=f"h3{h}")
            nc.gpsimd.memset(t[32:33, :], 1.0)
            h3h.append(t)
        # y staging [1, (b256, i2)] f32, one contiguous DMA per half
        ystage = consts.tile([1, 512], F32, tag="ystage")

        b0 = 0
        for ci, bc in enumerate(CHUNKS):
            half = b0 // 128
            nhf = bc // 16
            # xt layout [128, (k4, bc, 24)]: fully contiguous DMA
            xt = xpool.tile([128, 4 * 32 * 24], F16, tag="xt", name=f"xt{ci}")
            nc.sync.dma_start(
                xt[:, 0:4 * bc * 24].rearrange("p (k c) -> p k c", k=4),
                xT.rearrange("(k p) c -> p k c", k=4)
                [:, :, b0 * 24:(b0 + bc) * 24])
            xt4 = xt[:, 0:4 * bc * 24].rearrange("p (k b j) -> p k b j",
                                                 k=4, j=24)
            pss = [psA.tile([128, 384], F32, tag="psA", name=f"psA{ci}_{hf}")
                   for hf in range(nhf)]
            for k in range(4):  # weights loaded once per k
                for hf in range(nhf):
                    nc.tensor.matmul(
                        pss[hf][:, :],
                        gA_t[:, k * 128:(k + 1) * 128],
                        xt4[:, k, hf * 16:(hf + 1) * 16, :],
                        start=(k == 0), stop=(k == 3))
            # stage psum->sbuf (ScalarE) with (b,32) interleave; pad cols
            # already hold the delta pattern (bias rows of gB)
            sg = sg_bufs[ci % 3]
            sg3 = sg.rearrange("p (b j) -> p b j", j=32)
            for hf in range(nhf):
                nc.scalar.activation(
                    sg3[:, hf * 16:(hf + 1) * 16, 0:24],
                    pss[hf].rearrange("p (b j) -> p b j", j=24),
                    AF.Copy)
            # 32x32 stream transpose: [(g,u),(b,j)] -> [(g,j),(b,u)]
            tb = tpool.tile([128, 32 * 32], F16, tag="tb", name=f"tb{ci}")
            nc.vector.transpose(tb[:, 0:bc * 32], sg[:, 0:bc * 32])
            lb = b0 - half * 128
            dst = h1h[half][:, lb * 16:(lb + bc) * 16] \
                .rearrange("p (b v) -> p b v", v=16)
            for hf in range(nhf):
                # phase B: layer-1 left side + bias (pad rows x delta)
                pb = psB.tile([64, 512], F32, tag="psB", name=f"psB{ci}_{hf}")
                nc.tensor.matmul(pb[:, :], gB_t[:, :],
                                 tb[:, hf * 512:(hf + 1) * 512],
                                 start=True, stop=True)
                # relu + split (m3l) into h1: aligned half on ScalarE,
                # partition-shifted half on DVE
                pb3 = pb.rearrange("p (b u) -> p b u", u=32)
                d3 = dst[:, hf * 16:(hf + 1) * 16, :]
                nc.scalar.activation(d3[0:64], pb3[:, :, 0:16], AF.Relu)
                nc.vector.tensor_scalar_max(d3[64:128], pb3[:, :, 16:32], 0.0)
            b0 += bc

            if b0 % 128 == 0:
                # ---- layers 2/3 + log_softmax tail for this b128 half ----
                # L2: even/odd v alternate between PE column halves (out
                # partitions 0:64 / 64:128) so each LDWEIGHTS overlaps the
                # other half's matmul instead of serializing
                p2p = psT.tile([128, 128], F32, tag="p2p", name=f"p2p{half}")
                h1v = h1h[half].rearrange("p (b v) -> p v b", v=16)
                for v in range(16):
                    e = v % 2
                    nc.tensor.matmul(p2p[e * 64:(e + 1) * 64, :],
                                     g2_t[:, v * 64:(v + 1) * 64],
                                     h1v[:, v, :], start=(v < 2),
                                     stop=(v >= 14))
                s2a = spool.tile([64, 128], F32, tag=f"s2a{half}",
                                 name=f"s2a{half}")
                nc.scalar.activation(s2a[:, :], p2p[0:64, :], AF.Copy)
                s2s = spool.tile([64, 128], F32, tag=f"s2s{half}",
                                 name=f"s2s{half}")
                nc.vector.tensor_tensor(s2s[:, :], s2a[:, :],
                                        p2p[64:128, :], op=ADD)
                h2 = spool.tile([64, 128], F16, tag=f"h2{half}",
                                name=f"h2_{half}")
                nc.scalar.activation(h2[:, :], s2s[:, :], AF.Relu,
                                     bias=b2_t[:, 0:1])
                p3 = psT.tile([32, 128], F32, tag="pt", name=f"p3_{half}")
                nc.tensor.matmul(p3[:, :], g3_t[:, :], h2[:, :],
                                 start=True, stop=True)
                nc.scalar.activation(h3h[half][0:32, :], p3[:, :], AF.Relu,
                                     bias=b3_t[:, 0:1])
                # logit diff (K=33 includes the bld ones-row)
                pd = psT.tile([1, 128], F32, tag="pt", name=f"pd_{half}")
                nc.tensor.matmul(pd[:, :], wd_t[:, :], h3h[half][:, :],
                                 start=True, stop=True)
                # log_softmax = [-softplus(d), d - softplus(d)] with
                # softplus(d) = relu(d) - ln(1-s), s = sigmoid(-|d|), and
                # -ln(1-s) ~= s + s^2/2 + s^3/3 + s^4/4 + s^5/5 (|err|<.5%)
                #           = s*(1 + s2/3 + s4/5) + s2*(1/2 + s2/4)
                aT = spool.tile([1, 128], F32, tag=f"aT{half}",
                                name=f"aT{half}")
                nc.scalar.activation(aT[:, :], pd[:, :], AF.Abs)
                sS = spool.tile([1, 128], F32, tag=f"sS{half}",
                                name=f"sS{half}")
                nc.scalar.activation(sS[:, :], aT[:, :], AF.Sigmoid,
                                     scale=-1.0)
                s2 = spool.tile([1, 128], F32, tag=f"s2{half}",
                                name=f"s2{half}")
                nc.scalar.activation(s2[:, :], sS[:, :], AF.Square)
                s4 = spool.tile([1, 128], F32, tag=f"s4{half}",
                                name=f"s4{half}")
                nc.scalar.activation(s4[:, :], s2[:, :], AF.Square)
                rl = spool.tile([1, 128], F32, tag=f"rl{half}",
                                name=f"rl{half}")
                nc.scalar.activation(rl[:, :], pd[:, :], AF.Relu)
                qa = spool.tile([1, 128], F32, tag=f"qa{half}",
                                name=f"qa{half}")
                qb = spool.tile([1, 128], F32, tag=f"qb{half}",
                                name=f"qb{half}")
                qc = spool.tile([1, 128], F32, tag=f"qc{half}",
                                name=f"qc{half}")
                # qa = 1 + s2/3 + s4/5 ; qb = 1/2 + s2/4
                nc.vector.tensor_scalar(qa[:, :], s4[:, :], 0.2, 1.0,
                                        MUL, ADD)
                nc.vector.scalar_tensor_tensor(qa[:, :], s2[:, :], 1.0 / 3,
                                               qa[:, :], MUL, ADD)
                nc.vector.tensor_scalar(qb[:, :], s2[:, :], 0.25, 0.5,
                                        MUL, ADD)
                # qc = s*qa + s2*qb = -ln(1-s);  then y0 = -(qc + relu(d))
                nc.vector.tensor_tensor(qc[:, :], sS[:, :], qa[:, :], op=MUL)
                nc.vector.tensor_tensor(qb[:, :], s2[:, :], qb[:, :], op=MUL)
                nc.vector.tensor_tensor(qc[:, :], qc[:, :], qb[:, :], op=ADD)
                yv = ystage.rearrange("p (b i) -> p b i", i=2)
                hb = half * 128
                nc.vector.scalar_tensor_tensor(
                    yv[:, hb:hb + 128, 0:1],
                    qc.rearrange("p (f o) -> p f o", o=1), -1.0,
                    rl.rearrange("p (f o) -> p f o", o=1),
                    MUL, SUB)
                nc.vector.tensor_tensor(
                    yv[:, hb:hb + 128, 1:2],
                    pd.rearrange("p (f o) -> p f o", o=1),
                    yv[:, hb:hb + 128, 0:1], op=ADD)
                nc.sync.dma_start(
                    y.rearrange("(h b) i -> h (b i)", h=2)[half:half + 1, :],
                    ystage[:, hb * 2:hb * 2 + 256])

    nc.compile()
    _prog_cache['nc'] = nc
    return nc


# ---------------------------------------------------------------------------
# Entry point
# ---------------------------------------------------------------------------
def kernel(**inputs):
    from concourse.bass_utils import run_bass_kernel_spmd

    H = _build_host_tensors(inputs)
    x = np.asarray(inputs['x'], np.float32)
    nc = _build_program()

    in_maps = []
    for c in range(NCORES):
        m = dict(H)
        m['xT'] = _make_xT(x[c * BLOC:(c + 1) * BLOC])
        in_maps.append(m)

    trace = bool(os.environ.get('KERNEL_TRACE'))
    tmpdir = None
    if trace:
        tmpdir = os.environ.get('KERNEL_TRACE_DIR') or None
        if tmpdir:
            os.makedirs(tmpdir, exist_ok=True)
    res = run_bass_kernel_spmd(nc, in_maps, list(range(NCORES)),
                               trace=trace, tmpdir=tmpdir)
    kernel.last_results = res
    out = np.concatenate([res.results[c]['y'] for c in range(NCORES)], axis=0)
    return out.astype(np.float32)


if __name__ == '__main__':
    # smoke test with random inputs shaped per spec
    rng = np.random.default_rng(0)
    shapes = {
        'x': (B, 3, 8, 8, 8, 8),
        'l1c0': (1, 8, 3, 3), 'l1c1': (3, 4, 8, 2), 'l1c2': (2, 4, 8, 2),
        'l1c3': (2, 4, 8, 2), 'l1c4': (2, 4, 8, 1), 'b1': (8, 4, 4, 4, 4),
        'l2c0': (1, 4, 8, 2), 'l2c1': (2, 2, 4, 2), 'l2c2': (2, 2, 4, 2),
        'l2c3': (2, 2, 4, 2), 'l2c4': (2, 2, 4, 1), 'b2': (4, 2, 2, 2, 2),
        'l3c0': (1, 2, 4, 2), 'l3c1': (2, 2, 2, 2), 'l3c2': (2, 2, 2, 2),
        'l3c3': (2, 2, 2, 2), 'l3c4': (2, 2, 2, 1), 'b3': (2, 2, 2, 2, 2),
        'W': (2, 32), 'bl': (2,),
    }
    ins = {k: rng.standard_normal(v).astype(np.float32) * 0.3
           for k, v in shapes.items()}
    print(kernel(**ins)[:4])


# revision 14
# speedup vs baseline: 1.1776x; 1.0596x over previous
"""Trainium2 Bass kernel for nn_BasicTT (TT-decomposed 3-layer MLP + log_softmax).

Strategy (8-way batch data parallelism, b=256 per core):
  Host prep (numpy):
    - Merge layer-1 TT cores 3,4,5 -> lhsT_A [K=512=(n3,n4,n5), 128=(r2,m3,m4,m5)]
    - Merge layer-1 cores 1,2 (+ layer-1 bias in pad rows) -> lhsT_B [128, 64]
    - Layer 2 and 3 TT weights densified: g2 [128,(v16,64)], g3 [64,32]
    - Final linear reduced to the logit difference d = (W[1]-W[0])@h3 + bld
      (bld folded as a 33rd row of the d-matmul against a ones-row of h3);
      log_softmax = [-softplus(d), d - softplus(d)]
    - x pre-transposed per core to xT [512, b*24] fp16; all consts in one
      fp16 DMA
  Device (per core), pipelined per batch chunk (b16 head/tail chunks for
  low DMA-completion latency at the kernel edges, b32 in the middle):
    - warmup matmuls on dummy data span the HAM window while the input
      DMA streams, so real work starts at full PE clock
    - Phase A: accumulating K=128 fp16 matmuls -> psum [128,(b16,24)]
    - ScalarE copies psum -> sg [128,(b,32)] (pad cols hold the persistent
      bias-delta pattern); one DVE 32x32 stream-transpose per chunk
    - Phase B: per-b16 matmul (bias via pad rows) -> psB [64,(b16,u32)]
    - relu-split into h1 [128,(b128,16)]: ScalarE takes the aligned half,
      DVE the partition-shifted half
    - per b128 half: L2 as 16 acc matmuls alternating between two PSUM
      banks (even/odd v) to avoid drain-to-fill serialization, summed on
      DVE -> relu+b2 -> L3 -> relu+b3 -> d-matmul -> sigmoid-series
      softplus tail -> contiguous y DMA
  Only Abs/Sigmoid/Square/Relu/Copy activations are used (single ACT
  table load, pinned to kernel start by a dummy Sigmoid).
"""
import os
import numpy as np

NCORES = 8
B = 2048
BLOC = B // NCORES  # 256
# chunk sizes: small edges (fast DMA completion), b32 middle
CHUNKS = [16, 16, 32, 32, 32, 32, 32, 32, 16, 16]
assert sum(CHUNKS) == BLOC

_prog_cache = {}


# ---------------------------------------------------------------------------
# Host-side weight preparation
# ---------------------------------------------------------------------------
def _tt_full_matrix(cores):
    """Dense matrix W [prod(m), prod(n)] of a TT layer, matching the
    reference tt_linear index convention."""
    n = 1
    for G in cores:
        n *= G.shape[2]
    x = np.eye(n)
    b = n
    z = x.reshape(b, 1, -1)
    for G in cores:
        r0, m, nn_, r1 = G.shape
        z4 = z.reshape(b, r0, nn_, -1)
        z = np.einsum('brns,rmnq->bqsm', z4, G).reshape(b, r1, -1)
    return z.reshape(b, -1).T


def _build_host_tensors(p):
    f64 = {k: np.asarray(v, np.float64) for k, v in p.items()}

    # G345 = l1c2 (r2,m3,n3,r3) * l1c3 (r3,m4,n4,r4) * l1c4 (r4,m5,n5,1)
    g34 = np.einsum('amcb,bndq->amncdq', f64['l1c2'], f64['l1c3'])
    g345 = np.einsum('amncdq,qpe->amnpcde', g34, f64['l1c4'][:, :, :, 0])
    # g345[r2,m3,m4,m5,n3,n4,n5] -> lhsT_A [(n3,n4,n5)=512, (r2,m3,m4,m5)=128]
    lhsT_A = g345.transpose(4, 5, 6, 0, 1, 2, 3).reshape(512, 128)
    gA = np.ascontiguousarray(
        lhsT_A.reshape(4, 128, 128).transpose(1, 0, 2).reshape(128, 512))

    # G12 = l1c0 (1,m1,n1,r1) * l1c1 (r1,m2,n2,r2) -> g12[n1,n2,r2,m1,m2]
    g12 = np.einsum('mar,rnbq->abqmn', f64['l1c0'][0], f64['l1c1'])
    b1 = f64['b1']  # (m1,m2,m3,m4,m5) = (8,4,4,4,4)
    lhsT_B = np.zeros((128, 64))
    for r2 in range(2):
        for m3h in range(2):
            g = r2 * 2 + m3h
            for n1 in range(3):
                for n2 in range(8):
                    j = n1 * 8 + n2
                    for m1 in range(8):
                        for m2 in range(4):
                            lhsT_B[g * 32 + j, m3h * 32 + m1 * 4 + m2] = \
                                g12[n1, n2, r2, m1, m2]
    # bias rows: row (g, 24+j') fires for u = 8g+j'
    for g in range(4):
        for jp in range(8):
            u = 8 * g + jp
            m3l, m4, m5 = u >> 4, (u >> 2) & 3, u & 3
            for m3h in range(2):
                for m1 in range(8):
                    for m2 in range(4):
                        m3 = m3h * 2 + m3l
                        lhsT_B[g * 32 + 24 + jp, m3h * 32 + m1 * 4 + m2] = \
                            b1[m1, m2, m3, m4, m5]
    # delta pattern for the sg pads, tiled over b32: row p=(g,u) has 1.0 at
    # pad col (b, j') iff u == 8g+j'
    dltrep = np.zeros((128, 8))
    for gg in range(4):
        for u in range(32):
            jp = u - 8 * gg
            if 0 <= jp < 8:
                dltrep[gg * 32 + u, jp] = 1.0
    dltrep = np.tile(dltrep, (1, 32))  # [128, 256]

    # dense layer 2/3
    W2 = _tt_full_matrix([f64['l2c0'], f64['l2c1'], f64['l2c2'],
                          f64['l2c3'], f64['l2c4']])  # [64, 2048]
    W3 = _tt_full_matrix([f64['l3c0'], f64['l3c1'], f64['l3c2'],
                          f64['l3c3'], f64['l3c4']])  # [32, 64]
    # g2 [128=(m3l,m3h,m1,m2), (v16, f64)]
    g2 = np.zeros((128, 16, 64))
    for pp in range(128):
        m3l, m3h = pp >> 6, (pp >> 5) & 1
        m1, m2 = (pp >> 2) & 7, pp & 3
        m3 = m3h * 2 + m3l
        for v in range(16):
            m4, m5 = v >> 2, v & 3
            flat = (((m1 * 4 + m2) * 4 + m3) * 4 + m4) * 4 + m5
            g2[pp, v, :] = W2[:, flat]
    g2 = g2.reshape(128, 1024)

    # consts fp16, two DMAs: cstA holds everything the chunk pipeline
    # needs (so phase A can start as soon as possible), cstB what the
    # per-half tail needs (L2/L3/logit weights)
    #   cstA: gA 0:512 | gB 512:576 | dltrep 576:832 | b2 832:833 | b3 833:834
    #   cstB: g2 0:1024 | g3 1024:1056 (rows 0:64) | wd 1056:1057 (rows 0:33)
    cstA = np.zeros((128, 834), np.float16)
    cstA[:, 0:512] = gA.astype(np.float16)
    cstA[:, 512:576] = lhsT_B.astype(np.float16)
    cstA[:, 576:832] = dltrep.astype(np.float16)
    cstA[0:64, 832:833] = f64['b2'].reshape(64, 1).astype(np.float16)
    cstA[0:32, 833:834] = f64['b3'].reshape(32, 1).astype(np.float16)
    cstB = np.zeros((128, 1057), np.float16)
    cstB[:, 0:1024] = g2.astype(np.float16)
    cstB[0:64, 1024:1056] = W3.T.astype(np.float16)
    cstB[0:32, 1056:1057] = (f64['W'][1] - f64['W'][0]).reshape(32, 1) \
        .astype(np.float16)
    cstB[32, 1056] = np.float16(f64['bl'][1] - f64['bl'][0])
    return dict(cstA=cstA, cstB=cstB)


def _make_xT(x_core):
    b = x_core.shape[0]
    xr = np.asarray(x_core, np.float32).reshape(b, 24, 512)
    xt = np.ascontiguousarray(xr.transpose(2, 0, 1).reshape(512, b * 24))
    return xt.astype(np.float16)


# ---------------------------------------------------------------------------
# Device program
# ---------------------------------------------------------------------------
def _build_program():
    if 'nc' in _prog_cache:
        return _prog_cache['nc']
    from contextlib import ExitStack
    import concourse.bacc as bacc
    import concourse.mybir as mybir
    import concourse.tile as tile

    F16 = mybir.dt.float16
    F32 = mybir.dt.float32
    AF = mybir.ActivationFunctionType
    MUL = mybir.AluOpType.mult
    ADD = mybir.AluOpType.add
    SUB = mybir.AluOpType.subtract

    nc = bacc.Bacc(None, target_bir_lowering=False)

    xT = nc.declare_dram_parameter("xT", [512, BLOC * 24], F16, isOutput=False)
    cstA = nc.declare_dram_parameter("cstA", [128, 834], F16, isOutput=False)
    cstB = nc.declare_dram_parameter("cstB", [128, 1057], F16, isOutput=False)
    y = nc.declare_dram_parameter("y", [BLOC, 2], F32, isOutput=True)

    with tile.TileContext(nc) as tc, ExitStack() as ctx:
        consts = ctx.enter_context(tc.tile_pool(name="consts", bufs=1))
        xpool = ctx.enter_context(tc.tile_pool(name="x", bufs=3))
        tpool = ctx.enter_context(tc.tile_pool(name="tb", bufs=3))
        h1pool = ctx.enter_context(tc.tile_pool(name="h1", bufs=1))
        spool = ctx.enter_context(tc.tile_pool(name="small", bufs=1))
        psA = ctx.enter_context(tc.tile_pool(name="psA", bufs=4, space="PSUM"))
        psB = ctx.enter_context(tc.tile_pool(name="psB", bufs=2, space="PSUM"))
        psT = ctx.enter_context(tc.tile_pool(name="psT", bufs=1, space="PSUM"))

        cA = consts.tile([128, 834], F16, tag="cstA")
        nc.sync.dma_start(cA[:, :], cstA[:, :])
        cB = consts.tile([128, 1057], F16, tag="cstB")
        gA_t = cA[:, 0:512]
        gB_t = cA[:, 512:576]
        dltrep_t = cA[:, 576:832]
        b2_t = cA[0:64, 832:833]
        b3_t = cA[0:32, 833:834]
        g2_t = cB[:, 0:1024]
        g3_t = cB[0:64, 1024:1056]
        wd_t = cB[0:33, 1056:1057]

        # PE warmup during the input-DMA window: dummy matmuls on a zeroed
        # tile span the HAM activity window so real work runs at 2.4 GHz.
        wrm = consts.tile([128, 512], F16, tag="wrm")
        nc.vector.memset(wrm[0:1, 0:4], 0.0)
        # dummy Sigmoid first: every activation used later (Abs, Sigmoid,
        # Square, Relu, Copy) lives in the sigmoid table, so this pins the
        # one and only ACT table load to kernel start
        sgd = consts.tile([1, 1], F32, tag="sgd")
        nc.scalar.activation(sgd[:, :], wrm[0:1, 0:1], AF.Sigmoid)
        for w in range(5):
            pw = psA.tile([128, 384], F32, tag="psA", name=f"warm{w}")
            nc.tensor.matmul(pw[:, :], wrm[:, 0:128], wrm[:, 0:384],
                             start=True, stop=True)

        # persistent staging buffers: pad cols (b, 24:32) hold the bias
        # delta pattern, written once; data cols rewritten every chunk
        sg_bufs = []
        for i in range(3):
            z = tpool.tile([128, 32 * 32], F16, tag=f"sg{i}", name=f"sg{i}")
            nc.vector.tensor_copy(
                z.rearrange("p (b j) -> p b j", j=32)[:, :, 24:32],
                dltrep_t.rearrange("p (b j) -> p b j", j=8))
            sg_bufs.append(z)

        # h1 per b128 half [128=(m3l,m3h,m1,m2), (b128, v16)]
        h1h = [h1pool.tile([128, 128 * 16], F16, tag=f"h1{h}", name=f"h1{h}")
               for h in range(2)]
        # h3 per half [33, 128]: row 32 is the ones-row that folds bld in
        h3h = []
        for h in range(2):
            t = spool.tile([33, 128], F16, tag=f"h3{h}", name=f"h3{h}")
            nc.gpsimd.memset(t[32:33, :], 1.0)
            h3h.append(t)
        # y staging [1, (b256, i2)] f32, one contiguous DMA per half
        ystage = consts.tile([1, 512], F32, tag="ystage")

        b0 = 0
        for ci, bc in enumerate(CHUNKS):
            half = b0 // 128
            nhf = bc // 16
            # xt layout [128, (k4, bc, 24)]: fully contiguous DMA
            xt = xpool.tile([128, 4 * 32 * 24], F16, tag="xt", name=f"xt{ci}")
            xTs = xT.rearrange("(k p) c -> p k c", k=4)
            if bc == 16:
                # k-split: 4 small DMAs so phase-A k-matmuls start on the
                # first quarter instead of waiting for the whole chunk
                for k in range(4):
                    nc.sync.dma_start(
                        xt[:, k * bc * 24:(k + 1) * bc * 24],
                        xTs[:, k, b0 * 24:(b0 + bc) * 24])
            else:
                nc.sync.dma_start(
                    xt[:, 0:4 * bc * 24].rearrange("p (k c) -> p k c", k=4),
                    xTs[:, :, b0 * 24:(b0 + bc) * 24])
            if ci == 2:
                nc.sync.dma_start(cB[:, :], cstB[:, :])
            xt4 = xt[:, 0:4 * bc * 24].rearrange("p (k b j) -> p k b j",
                                                 k=4, j=24)
            pss = [psA.tile([128, 384], F32, tag="psA", name=f"psA{ci}_{hf}")
                   for hf in range(nhf)]
            for k in range(4):  # weights loaded once per k
                for hf in range(nhf):
                    nc.tensor.matmul(
                        pss[hf][:, :],
                        gA_t[:, k * 128:(k + 1) * 128],
                        xt4[:, k, hf * 16:(hf + 1) * 16, :],
                        start=(k == 0), stop=(k == 3))
            # stage psum->sbuf (ScalarE) with (b,32) interleave; pad cols
            # already hold the delta pattern (bias rows of gB)
            sg = sg_bufs[ci % 3]
            sg3 = sg.rearrange("p (b j) -> p b j", j=32)
            for hf in range(nhf):
                nc.scalar.activation(
                    sg3[:, hf * 16:(hf + 1) * 16, 0:24],
                    pss[hf].rearrange("p (b j) -> p b j", j=24),
                    AF.Copy)
            # 32x32 stream transpose: [(g,u),(b,j)] -> [(g,j),(b,u)]
            tb = tpool.tile([128, 32 * 32], F16, tag="tb", name=f"tb{ci}")
            nc.vector.transpose(tb[:, 0:bc * 32], sg[:, 0:bc * 32])
            lb = b0 - half * 128
            dst = h1h[half][:, lb * 16:(lb + bc) * 16] \
                .rearrange("p (b v) -> p b v", v=16)
            for hf in range(nhf):
                # phase B: layer-1 left side + bias (pad rows x delta)
                pb = psB.tile([64, 512], F32, tag="psB", name=f"psB{ci}_{hf}")
                nc.tensor.matmul(pb[:, :], gB_t[:, :],
                                 tb[:, hf * 512:(hf + 1) * 512],
                                 start=True, stop=True)
                # relu + split (m3l) into h1: aligned half on ScalarE,
                # partition-shifted half on DVE
                pb3 = pb.rearrange("p (b u) -> p b u", u=32)
                d3 = dst[:, hf * 16:(hf + 1) * 16, :]
                nc.scalar.activation(d3[0:64], pb3[:, :, 0:16], AF.Relu)
                nc.vector.tensor_scalar_max(d3[64:128], pb3[:, :, 16:32], 0.0)
            b0 += bc

            if b0 % 128 == 0:
                # ---- layers 2/3 + log_softmax tail for this b128 half ----
                # L2: even/odd v alternate between PE column halves (out
                # partitions 0:64 / 64:128) so each LDWEIGHTS overlaps the
                # other half's matmul instead of serializing
                p2p = psT.tile([128, 128], F32, tag="p2p", name=f"p2p{half}")
                h1v = h1h[half].rearrange("p (b v) -> p v b", v=16)
                for v in range(16):
                    e = v % 2
                    nc.tensor.matmul(p2p[e * 64:(e + 1) * 64, :],
                                     g2_t[:, v * 64:(v + 1) * 64],
                                     h1v[:, v, :], start=(v < 2),
                                     stop=(v >= 14))
                s2a = spool.tile([64, 128], F32, tag=f"s2a{half}",
                                 name=f"s2a{half}")
                nc.scalar.activation(s2a[:, :], p2p[0:64, :], AF.Copy)
                s2s = spool.tile([64, 128], F32, tag=f"s2s{half}",
                                 name=f"s2s{half}")
                nc.vector.tensor_tensor(s2s[:, :], s2a[:, :],
                                        p2p[64:128, :], op=ADD)
                h2 = spool.tile([64, 128], F16, tag=f"h2{half}",
                                name=f"h2_{half}")
                nc.scalar.activation(h2[:, :], s2s[:, :], AF.Relu,
                                     bias=b2_t[:, 0:1])
                p3 = psT.tile([32, 128], F32, tag="pt", name=f"p3_{half}")
                nc.tensor.matmul(p3[:, :], g3_t[:, :], h2[:, :],
                                 start=True, stop=True)
                nc.scalar.activation(h3h[half][0:32, :], p3[:, :], AF.Relu,
                                     bias=b3_t[:, 0:1])
                # logit diff (K=33 includes the bld ones-row)
                pd = psT.tile([1, 128], F32, tag="pt", name=f"pd_{half}")
                nc.tensor.matmul(pd[:, :], wd_t[:, :], h3h[half][:, :],
                                 start=True, stop=True)
                # log_softmax = [-softplus(d), d - softplus(d)] with
                # softplus(d) = relu(d) - ln(1-s), s = sigmoid(-|d|), and
                # -ln(1-s) ~= s + s^2/2 + s^3/3 + s^4/4 + s^5/5 (|err|<.5%)
                #           = s*(1 + s2/3 + s4/5) + s2*(1/2 + s2/4)
                aT = spool.tile([1, 128], F32, tag=f"aT{half}",
                                name=f"aT{half}")
                nc.scalar.activation(aT[:, :], pd[:, :], AF.Abs)
                sS = spool.tile([1, 128], F32, tag=f"sS{half}",
                                name=f"sS{half}")
                nc.scalar.activation(sS[:, :], aT[:, :], AF.Sigmoid,
                                     scale=-1.0)
                s2 = spool.tile([1, 128], F32, tag=f"s2{half}",
                                name=f"s2{half}")
                nc.scalar.activation(s2[:, :], sS[:, :], AF.Square)
                s4 = spool.tile([1, 128], F32, tag=f"s4{half}",
                                name=f"s4{half}")
                nc.scalar.activation(s4[:, :], s2[:, :], AF.Square)
                rl = spool.tile([1, 128], F32, tag=f"rl{half}",
                                name=f"rl{half}")
                nc.scalar.activation(rl[:, :], pd[:, :], AF.Relu)
                qa = spool.tile([1, 128], F32, tag=f"qa{half}",
                                name=f"qa{half}")
                qb = spool.tile([1, 128], F32, tag=f"qb{half}",
                                name=f"qb{half}")
                qc = spool.tile([1, 128], F32, tag=f"qc{half}",
                                name=f"qc{half}")
                # qa = 1 + s2/3 + s4/5 ; qb = 1/2 + s2/4
                nc.vector.tensor_scalar(qa[:, :], s4[:, :], 0.2, 1.0,
                                        MUL, ADD)
                nc.vector.scalar_tensor_tensor(qa[:, :], s2[:, :], 1.0 / 3,
                                               qa[:, :], MUL, ADD)
                nc.vector.tensor_scalar(qb[:, :], s2[:, :], 0.25, 0.5,
                                        MUL, ADD)
                # qc = s*qa + s2*qb = -ln(1-s);  then y0 = -(qc + relu(d))
                nc.vector.tensor_tensor(qc[:, :], sS[:, :], qa[:, :], op=MUL)
                nc.vector.tensor_tensor(qb[:, :], s2[:, :], qb[:, :], op=MUL)
                nc.vector.tensor_tensor(qc[:, :], qc[:, :], qb[:, :], op=ADD)
                yv = ystage.rearrange("p (b i) -> p b i", i=2)
                hb = half * 128
                nc.vector.scalar_tensor_tensor(
                    yv[:, hb:hb + 128, 0:1],
                    qc.rearrange("p (f o) -> p f o", o=1), -1.0,
                    rl.rearrange("p (f o) -> p f o", o=1),
                    MUL, SUB)
                nc.vector.tensor_tensor(
                    yv[:, hb:hb + 128, 1:2],
                    pd.rearrange("p (f o) -> p f o", o=1),
                    yv[:, hb:hb + 128, 0:1], op=ADD)
                nc.sync.dma_start(
                    y.rearrange("(h b) i -> h (b i)", h=2)[half:half + 1, :],
                    ystage[:, hb * 2:hb * 2 + 256])

    nc.compile()
    _prog_cache['nc'] = nc
    return nc


# ---------------------------------------------------------------------------
# Entry point
# ---------------------------------------------------------------------------
def kernel(**inputs):
    from concourse.bass_utils import run_bass_kernel_spmd

    H = _build_host_tensors(inputs)
    x = np.asarray(inputs['x'], np.float32)
    nc = _build_program()

    in_maps = []
    for c in range(NCORES):
        m = dict(H)
        m['xT'] = _make_xT(x[c * BLOC:(c + 1) * BLOC])
        in_maps.append(m)

    trace = bool(os.environ.get('KERNEL_TRACE'))
    tmpdir = None
    if trace:
        tmpdir = os.environ.get('KERNEL_TRACE_DIR') or None
        if tmpdir:
            os.makedirs(tmpdir, exist_ok=True)
    res = run_bass_kernel_spmd(nc, in_maps, list(range(NCORES)),
                               trace=trace, tmpdir=tmpdir)
    kernel.last_results = res
    out = np.concatenate([res.results[c]['y'] for c in range(NCORES)], axis=0)
    return out.astype(np.float32)


if __name__ == '__main__':
    # smoke test with random inputs shaped per spec
    rng = np.random.default_rng(0)
    shapes = {
        'x': (B, 3, 8, 8, 8, 8),
        'l1c0': (1, 8, 3, 3), 'l1c1': (3, 4, 8, 2), 'l1c2': (2, 4, 8, 2),
        'l1c3': (2, 4, 8, 2), 'l1c4': (2, 4, 8, 1), 'b1': (8, 4, 4, 4, 4),
        'l2c0': (1, 4, 8, 2), 'l2c1': (2, 2, 4, 2), 'l2c2': (2, 2, 4, 2),
        'l2c3': (2, 2, 4, 2), 'l2c4': (2, 2, 4, 1), 'b2': (4, 2, 2, 2, 2),
        'l3c0': (1, 2, 4, 2), 'l3c1': (2, 2, 2, 2), 'l3c2': (2, 2, 2, 2),
        'l3c3': (2, 2, 2, 2), 'l3c4': (2, 2, 2, 1), 'b3': (2, 2, 2, 2, 2),
        'W': (2, 32), 'bl': (2,),
    }
    ins = {k: rng.standard_normal(v).astype(np.float32) * 0.3
           for k, v in shapes.items()}
    print(kernel(**ins)[:4])
